# revision 64
# baseline (speedup 1.0000x reference)
"""Distributed causal attention (qkv proj + RoPE + SDPA + out proj) on 8 trn2 cores.

Sharding: data-parallel over batch (B=2), tensor-parallel over heads
(12 heads -> 4 groups of 3). Core c handles batch c//4, heads 3*(c%4)..3*(c%4)+2.
Each core computes a partial output x_b @ Wqkv_heads -> attention -> @ Wo_rows;
the host sums the 4 head-group partials per batch (fp32 accumulation of bf16
partials).

Host packs every input partition-contiguous ([128, big-free-dim]) so each
dma_start is ~128 large descriptors (fast HWDGE issue), and the x / wqkv loads
are split so the first projection matmul can start ~1.5us in.

Device schedule: software-pipelined attention over units
  [(01,0), (01,1), (2,0), (01,2), (2,1), (01,3), (2,2), (2,3)]
(unit (01,j) = heads 0+1 of query block j via row-packed K=64 matmuls; (2,j) =
head 2 alternating duplicated row halves).  Only the first q/k query-block
projections run up front (fused k-chains on the two pp psum slots); the
remaining projection runs as per-query-block pieces interleaved into the
units' exp-paced PE gaps from a dedicated double-buffered 1-bank psum ring,
each piece landing one window before its first consumer (emission order is a
CORRECTNESS constraint: a consumer emitted before its producer gets no
dependency edge).  Leftover PV/norm/wo steps carry into the next unit's window
instead of draining at unit boundaries, so next-unit score matmuls are not
parked behind wo matmuls waiting on the norm chain.  Softmax normalization:
DVE copy of the PSUM denominator row to partition 0 (reciprocal_approx_fast
and partition_broadcast both ignore AP base partitions), reciprocal, gpsimd
partition_broadcast, and one fused psum-x-bcast multiply into the bf16
attention-out tile.  Block 3's output projection runs in the scores PSUM ring
(free by then -> double-buffered tail); block 0's in the pp ring; blocks 1-2
on the wp ring in later units' windows.
"""
import numpy as np

B, T, C = 2, 2048, 768
H, DH = 12, 64
HPC = 3            # heads per core
NC_ = 8            # cores
QB = 512           # query block
KC = 128           # key chunk
NJ = T // QB       # 4 query blocks
NKC = T // KC      # 16 key chunks
SCALE = 1.0 / float(np.sqrt(DH))

_prog = None


def _build():
    import concourse.bass as bass
    import concourse.tile as tile
    from concourse import bacc, mybir

    f32 = mybir.dt.float32
    bf16 = mybir.dt.bfloat16
    Exp = mybir.ActivationFunctionType.Exp
    is_ge = mybir.AluOpType.is_ge

    nc = bacc.Bacc("TRN2", target_bir_lowering=False, debug=False)

    # packed, partition-contiguous inputs (see _host_prep)
    xp_p = nc.declare_dram_parameter("xp", [128, 6 * T], bf16, isOutput=False)
    wq_p = nc.declare_dram_parameter("wqp", [128, 6 * 576], bf16, isOutput=False)
    tb_p = nc.declare_dram_parameter("tbp", [128, 2 * T], bf16, isOutput=False)
    wo_p = nc.declare_dram_parameter("wop", [128, 1536], bf16, isOutput=False)
    out_p = nc.declare_dram_parameter("out", [T, C], bf16, isOutput=True)

    with tile.TileContext(nc) as tc:
        with tc.tile_pool(name="persist", bufs=1) as persist:
            mask = persist.tile([128, 4, QB], bf16, tag="mask")
            q01 = persist.tile([128, T], bf16, tag="q01")
            k01 = persist.tile([128, T], bf16, tag="k01")
            qk2 = persist.tile([128, T], bf16, tag="qk2")   # rows 0:64 q2, 64:128 dup
            k2al = persist.tile([128, T], bf16, tag="k2al")  # rows 0:64 k2, 64:128 dup
            vones = persist.tile([128, NKC, HPC, DH + 1], bf16, tag="vones")
            warm = persist.tile([1, 8], f32, tag="warm")
            wq = persist.tile([128, 6, 576], bf16, tag="wq")
            xts = persist.tile([128, 6, T], bf16, tag="xts")
            # cols 0:T cos, T:2T sin (sin rows swapped+sign-folded so the
            # rope swap-muls read in0/in1 at the same base partition)
            tbl = persist.tile([128, 2 * T], bf16, tag="tbl")
            wo_t = persist.tile([128, 1536], bf16, tag="wo_t")
            ropetmp = persist.tile([128, T // 2], bf16, tag="ropetmp")

            # preload the exp table set before anything else on ACT
            nc.vector.memset(warm, 0.0)
            nc.scalar.activation(out=warm, in_=warm, func=Exp, scale=1.0)
            # ones column of vones (for the fused softmax denominator)
            nc.gpsimd.memset(vones[:, :, :, DH:DH + 1], 1.0)
            # causal mask for the diagonal 4-chunk group:
            # mask[k, u, q'] = 1 if q' >= k + 128*u else 0
            nc.gpsimd.memset(mask, 1.0)
            nc.gpsimd.affine_select(
                out=mask, in_=mask,
                compare_op=is_ge, fill=0.0, base=0,
                pattern=[[-KC, 4], [1, QB]], channel_multiplier=-1)

            # --- input DMAs ---
            # Everything on ONE HWDGE ring in strict priority order: in-flight
            # DMAs share SDMA bandwidth round-robin across rings, so a second
            # ring would starve the critical-path loads.  Within one ring,
            # transfers complete FIFO.
            xr = xp_p.rearrange("p (k t) -> p k t", k=6)
            h0 = slice(0, T // 2)
            h1 = slice(T // 2, T)
            nc.sync.dma_start(out=wq[:, 0:2, :], in_=wq_p[:, 0:2 * 576])
            nc.sync.dma_start(out=xts[:, 0:2, h0], in_=xr[:, 0:2, h0])
            nc.sync.dma_start(out=wq[:, 2:6, :], in_=wq_p[:, 2 * 576:])
            nc.sync.dma_start(out=xts[:, 2:4, h0], in_=xr[:, 2:4, h0])
            nc.sync.dma_start(out=xts[:, 4:6, h0], in_=xr[:, 4:6, h0])
            nc.sync.dma_start(out=tbl[:, h0], in_=tb_p[:, h0])
            nc.sync.dma_start(out=tbl[:, T + h0.start:T + h0.stop],
                              in_=tb_p[:, T + h0.start:T + h0.stop])
            nc.sync.dma_start(out=tbl[:, h1], in_=tb_p[:, h1])
            nc.sync.dma_start(out=tbl[:, T + h1.start:T + h1.stop],
                              in_=tb_p[:, T + h1.start:T + h1.stop])
            for kp in range(3):
                nc.sync.dma_start(out=xts[:, 2 * kp:2 * kp + 2, h1],
                                  in_=xr[:, 2 * kp:2 * kp + 2, h1])
            nc.sync.dma_start(out=wo_t, in_=wo_p[:])

            wo01 = wo_t[:, 0:C]
            wo2 = wo_t[0:64, C:2 * C]
            cosT = tbl[:, 0:T]
            sinT = tbl[:, T:2 * T]

            def rope(X, out_q, out_k, sl):
                """RoPE X[:, sl] (2 blocks of 64 rows) in place, except that
                rows 64:128 may go to out_k (for q2k2 -> k2al)."""
                w = sl.stop - sl.start
                tp = ropetmp[:, 0:w]
                nc.vector.tensor_mul(tp[0:32], X[32:64, sl], sinT[32:64, sl])
                nc.vector.tensor_mul(tp[32:64], X[0:32, sl], sinT[0:32, sl])
                nc.vector.tensor_mul(tp[64:96], X[96:128, sl], sinT[96:128, sl])
                nc.vector.tensor_mul(tp[96:128], X[64:96, sl], sinT[64:96, sl])
                nc.vector.tensor_mul(X[:, sl], X[:, sl], cosT[:, sl])
                if out_k is None:
                    nc.vector.tensor_add(X[:, sl], X[:, sl], tp)
                else:
                    nc.vector.tensor_add(out_q[0:64, sl], X[0:64, sl], tp[0:64])
                    nc.vector.tensor_add(out_k[0:64, sl], X[64:128, sl], tp[64:128])

            # attention-phase state + pools (created up front: projection
            # shares the PSUM rings)
            with tc.tile_pool(name="phaseB", bufs=1) as pb, \
                 tc.tile_pool(name="bct", bufs=2) as bcp, \
                 tc.tile_pool(name="ostage", bufs=4) as osp:
                expts = [pb.tile([128, 2, NKC, QB], bf16, name=f"expt{i}",
                                 tag=f"expt{i}") for i in range(3)]
                outt01 = pb.tile([128, T], bf16, tag="outt01")
                outt2 = pb.tile([64, T], bf16, tag="outt2")
                # denom staging + reciprocal both live at partition 0:
                # reciprocal_approx_fast / partition_broadcast ignore AP base
                # partitions, so everything in this path must start at 0
                dstg = pb.tile([1, HPC * QB], f32, tag="dstg")
                recd = pb.tile([1, HPC * QB], f32, tag="recd")

                def tgt_of(h):
                    return outt01[0:64] if h == 0 else (
                        outt01[64:128] if h == 1 else outt2[0:64])

                with tc.tile_pool(name="sc", bufs=2, space="PSUM") as scp, \
                     tc.tile_pool(name="pv", bufs=2, space="PSUM") as pvp:
                    pools = {}

                    # q/k projection piece: M-tile m of qkvT = wqkv cols
                    # [128m, 128m+128), one query block at a time (1 psum
                    # bank -> double-buffered in the pp ring)
                    def proj_qk(m, X, out_q=None, out_k=None, qb=0,
                                cp_eng="s"):
                        pool = pools["fill"]
                        ps = pool.tile([128, QB], f32, tag="pp",
                                       name=f"pp{m}_{qb}")
                        for k in range(6):
                            nc.tensor.matmul(
                                ps,
                                lhsT=wq[:, k, m * 128:(m + 1) * 128],
                                rhs=xts[:, k, qb * QB:(qb + 1) * QB],
                                start=(k == 0), stop=(k == 5))
                        sl = slice(qb * QB, (qb + 1) * QB)
                        if cp_eng == "s":
                            nc.scalar.copy(X[:, sl], ps)
                        else:
                            nc.vector.tensor_copy(X[:, sl], ps)
                        rope(X, out_q, out_k, sl)

                    def vproj(t, ce="s"):
                        if t < 12:
                            pool, shape, tg = pools["fill"], [128, QB], "pp"
                        else:
                            pool, shape, tg = pools["wp"], [128, 2 * QB], "wp"
                        ps = pool.tile(shape, f32, tag=tg, name=f"vp{t}")
                        for k in range(6):
                            nc.tensor.matmul(
                                ps[:, 0:192],
                                lhsT=xts[:, k, t * 128:(t + 1) * 128],
                                rhs=wq[:, k, 384:576],
                                start=(k == 0), stop=(k == 5))
                        src = ps[:, 0:192].rearrange("p (h d) -> p h d", h=HPC)
                        if ce == "s":
                            nc.scalar.copy(vones[:, t, :, 0:DH], src)
                        else:
                            nc.vector.tensor_copy(vones[:, t, :, 0:DH], src)

                    pool_tag = {id(scp): "sc"}

                    def dup_qb(qb):
                        sl = slice(qb * QB, (qb + 1) * QB)
                        nc.vector.tensor_copy(qk2[64:128, sl], qk2[0:64, sl])
                        nc.vector.tensor_copy(k2al[64:128, sl], k2al[0:64, sl])

                    def s_steps(unit, expt):
                        """One closure per 2-bank scores psum group: 2 MMs +
                        exp (+ gpsimd causal mask for diagonal groups)."""
                        hh, j = unit
                        qsl = slice(j * QB, (j + 1) * QB)
                        steps = []
                        if hh == "01":
                            def grp01(c):
                                sc = scp.tile([128, 2 * QB], f32, tag="sc",
                                              name=f"sc01_{j}_{c}")
                                nc.tensor.matmul(
                                    sc[:, 0:QB],
                                    lhsT=k01[0:64, c * KC:(c + 1) * KC],
                                    rhs=q01[0:64, qsl],
                                    start=True, stop=True)
                                nc.tensor.matmul(
                                    sc[:, QB:2 * QB],
                                    lhsT=k01[64:128, c * KC:(c + 1) * KC],
                                    rhs=q01[64:128, qsl],
                                    start=True, stop=True)
                                nc.scalar.activation(
                                    out=expt[:, :, c, :],
                                    in_=sc.rearrange("p (hh q) -> p hh q", hh=2),
                                    func=Exp, scale=SCALE)
                                if c >= 4 * j:
                                    u = c - 4 * j
                                    m2 = mask[:, u, :]
                                    m2b = bass.AP(
                                        tensor=m2.tensor, offset=m2.offset,
                                        ap=[list(m2.ap[0]), [0, 2],
                                            list(m2.ap[1])])
                                    nc.vector.tensor_mul(
                                        expt[:, :, c, :],
                                        expt[:, :, c, :], m2b)
                            for c in range(4 * (j + 1)):
                                steps.append(lambda c=c: grp01(c))
                        else:
                            def grp2(g):
                                c0 = 2 * g
                                sc = scp.tile([128, 2 * QB], f32, tag="sc",
                                              name=f"sc2_{j}_{g}")
                                for uu in range(2):
                                    c = c0 + uu
                                    lo = c % 2 == 0
                                    nc.tensor.matmul(
                                        sc[:, uu * QB:(uu + 1) * QB],
                                        lhsT=k2al[0:64, c * KC:(c + 1) * KC] if lo
                                        else k2al[64:128, c * KC:(c + 1) * KC],
                                        rhs=qk2[0:64, qsl] if lo else qk2[64:128, qsl],
                                        start=True, stop=True)
                                nc.scalar.activation(
                                    out=expt[:, 0, c0:c0 + 2, :],
                                    in_=sc.rearrange("p (u q) -> p u q", u=2),
                                    func=Exp, scale=SCALE)
                                if c0 >= 4 * j:
                                    u0 = c0 - 4 * j
                                    nc.vector.tensor_mul(
                                        expt[:, 0, c0:c0 + 2, :],
                                        expt[:, 0, c0:c0 + 2, :],
                                        mask[:, u0:u0 + 2, :])
                            for g in range(2 * (j + 1)):
                                steps.append(lambda g=g: grp2(g))
                        return steps

                    def p_steps(unit, expt):
                        """PV matmul chunk-steps, then the fused
                        normalize, then (after the '2' unit) the block's
                        output projection."""
                        hh, j = unit
                        nch = 4 * (j + 1)
                        heads = [(0, 0), (1, 1)] if hh == "01" else [(2, 0)]
                        pos = {}
                        steps = []

                        def setup():
                            for h, _ in heads:
                                pos[h] = pvp.tile([128, QB], f32, tag="pv",
                                                  name=f"po_{h}_{j}")

                        def chunk(c):
                            for h, hh_slot in heads:
                                nc.tensor.matmul(
                                    pos[h][0:DH + 1, :],
                                    lhsT=vones[:, c, h, :],
                                    rhs=expt[:, hh_slot, c, :],
                                    start=(c == 0), stop=(c == nch - 1))

                        steps.append(setup)
                        for c0 in range(0, nch, 2):
                            def two(c0=c0):
                                chunk(c0)
                                chunk(c0 + 1)
                            steps.append(two)

                        def norm_recip():
                            usl = slice(heads[0][0] * QB,
                                        (heads[-1][0] + 1) * QB)
                            for h, _ in heads:
                                nc.vector.tensor_copy(
                                    dstg[0:1, h * QB:(h + 1) * QB],
                                    pos[h][DH:DH + 1, :])
                            with nc.allow_low_precision(
                                    reason="softmax denom reciprocal"):
                                nc.vector.reciprocal_approx_fast(
                                    out=recd[0:1, usl], in_=dstg[0:1, usl])

                        def norm_mul(h):
                            bct = bcp.tile([128, QB], f32, tag="bct",
                                           name=f"bct_{h}_{j}")
                            nc.gpsimd.partition_broadcast(
                                bct[0:64, :],
                                recd[0:1, h * QB:(h + 1) * QB])
                            tgt = tgt_of(h)
                            sl = slice(j * QB, (j + 1) * QB)
                            nc.vector.tensor_mul(
                                tgt[:, sl], pos[h][0:DH, :], bct[0:64, :])

                        fin_steps = [norm_recip]
                        for h, _ in heads:
                            fin_steps.append(lambda h=h: norm_mul(h))

                        if hh == "2":
                            pws = {}

                            def wo_mm(qq):
                                q = j * 4 + qq
                                if j == 0:
                                    # window 3 runs inside the pp scope: use a
                                    # pair of its 1-bank tiles
                                    pwA = pools["fill"].tile(
                                        [128, QB], f32, tag="pp",
                                        name=f"pwA_{q}")
                                    pwB = pools["fill"].tile(
                                        [128, QB], f32, tag="pp",
                                        name=f"pwB_{q}")
                                    parts = ((pwA[:, 0:512], 0, 512),
                                             (pwB[:, 0:256], 512, 768))
                                    pws[qq] = (pwA, pwB)
                                else:
                                    wo_pool = scp if j == 3 else pools["wp"]
                                    pw = wo_pool.tile(
                                        [128, 2 * QB], f32,
                                        tag=pool_tag[id(wo_pool)],
                                        name=f"pw_{q}")
                                    parts = ((pw[:, 0:512], 0, 512),
                                             (pw[:, 512:768], 512, 768))
                                    pws[qq] = (pw,)
                                for (dst, n0, n1) in parts:
                                    nc.tensor.matmul(
                                        dst,
                                        lhsT=outt01[:, q * 128:(q + 1) * 128],
                                        rhs=wo01[:, n0:n1],
                                        start=True, stop=False)
                                    nc.tensor.matmul(
                                        dst,
                                        lhsT=outt2[:, q * 128:(q + 1) * 128],
                                        rhs=wo2[:, n0:n1],
                                        start=False, stop=True)

                            def wo_out(qq):
                                q = j * 4 + qq
                                ot = osp.tile([128, C], bf16, tag="ot",
                                              name=f"ot_{q}")
                                eng = (nc.scalar.copy if qq % 2 == 0
                                       else nc.vector.tensor_copy)
                                if len(pws[qq]) == 2:
                                    pwA, pwB = pws[qq]
                                    eng(ot[:, 0:512], pwA[:, 0:512])
                                    eng(ot[:, 512:768], pwB[:, 0:256])
                                else:
                                    eng(ot, pws[qq][0][:, 0:C])
                                # always the sync ring: a DIRECT2D on the
                                # scalar queue costs ACT descriptor-gen time
                                # inside the exp FIFO (and can head-of-line
                                # block it); sync idles mid-phase
                                nc.sync.dma_start(
                                    out=out_p[q * 128:(q + 1) * 128, :], in_=ot)
                            for qq in range(4):
                                fin_steps.append(lambda qq=qq: wo_mm(qq))
                                fin_steps.append(lambda qq=qq: wo_out(qq))
                        return steps, fin_steps

                    # remaining projection work, interleaved into the
                    # units' exp-paced windows (emission order = PE priority);
                    # all of it runs in the dedicated 1-bank-double-buffered
                    # "pp" psum ring so it never steals the scores buffers.
                    # Each entry lands one window before its first consumer.
                    def fq(m, qb):
                        if m == 2:
                            return lambda: proj_qk(2, qk2, out_q=qk2,
                                                   out_k=k2al, qb=qb)
                        X = q01 if m == 0 else k01
                        return lambda: proj_qk(m, X, qb=qb)
                    extras = {
                        0: [fq(0, 1), fq(1, 1), fq(0, 2),
                            lambda: vproj(0, "s"), lambda: vproj(1, "s")],
                        1: [fq(2, 0), lambda: dup_qb(0), fq(1, 2),
                            fq(2, 1), lambda: dup_qb(1),
                            lambda: vproj(2), lambda: vproj(3)],
                        2: [lambda: vproj(4), lambda: vproj(5),
                            lambda: vproj(6), lambda: vproj(7)],
                        3: [fq(0, 3), fq(1, 3), lambda: vproj(8),
                            lambda: vproj(9), lambda: vproj(10),
                            lambda: vproj(11)],
                        4: [fq(2, 2), lambda: dup_qb(2),
                            fq(2, 3), lambda: dup_qb(3)],
                        5: [lambda: vproj(12), lambda: vproj(13),
                            lambda: vproj(14), lambda: vproj(15)],
                    }

                    units = [("01", 0), ("01", 1), ("2", 0), ("01", 2),
                             ("2", 1), ("01", 3), ("2", 2), ("2", 3)]

                    # lag-1 pipeline, interleaved at step granularity: PE runs
                    # the previous unit's PV/Wo steps (plus leftover
                    # projection) in the gaps between this unit's score groups
                    state = {"pv": [], "fin": [], "fin_hold": []}

                    def run_unit(i, u):
                        last = i == len(units) - 1
                        S = s_steps(u, expts[i % 3])
                        # extras BEFORE carried steps: a same-window consumer
                        # emitted ahead of its producer gets no dependency
                        # edge at all (Tile cannot see future writes) -> race.
                        # fin(i-2) precedes pv(i-1) so the pvp ring frees
                        # before the next setup allocates.
                        work = extras.get(i, []) + state["fin"] + state["pv"]
                        state["fin"] = state["fin_hold"]
                        state["fin_hold"] = []
                        if last:
                            opv, ofin = p_steps(u, expts[i % 3])
                            own = opv + ofin
                        done = 0
                        own_done = 0
                        for gi, s in enumerate(S):
                            s()
                            want = ((gi + 1) * len(work)) // len(S)
                            while done < want:
                                work[done]()
                                done += 1
                            if last and gi >= 2:
                                # own[0] is setup; chunk-step k is own[1+k]
                                while own_done < min(gi - 1, len(S) - 1) + 1:
                                    own[own_done]()
                                    own_done += 1
                        while done < len(work):
                            work[done]()
                            done += 1
                        if last:
                            state["pv"] = own[own_done:]
                            return
                        pv, fin = p_steps(u, expts[i % 3])
                        if i < 4:
                            # defer the norm+wo part one extra window: its
                            # deps (full PV accumulation) resolve late, and
                            # emitting it at a window end parks the next
                            # unit's score matmuls behind it in the PE FIFO
                            state["pv"] = pv
                            state["fin_hold"] = fin
                        else:
                            state["pv"] = pv + fin

                    # scope A: projection psum ring alive through units 0-4
                    with tc.tile_pool(name="pp", bufs=2, space="PSUM") as ppp:
                        pools["fill"] = ppp
                        pool_tag[id(ppp)] = "pp"
                        # fused first pieces: both are paced by the same x
                        # DMAs, so interleave their k-chains on the two pp
                        # slots instead of running them back-to-back
                        ps0 = ppp.tile([128, QB], f32, tag="pp", name="pp0_f")
                        ps1 = ppp.tile([128, QB], f32, tag="pp", name="pp1_f")
                        for k in range(6):
                            for m, ps in ((0, ps0), (1, ps1)):
                                nc.tensor.matmul(
                                    ps,
                                    lhsT=wq[:, k, m * 128:(m + 1) * 128],
                                    rhs=xts[:, k, 0:QB],
                                    start=(k == 0), stop=(k == 5))
                        nc.scalar.copy(q01[:, 0:QB], ps0)
                        nc.scalar.copy(k01[:, 0:QB], ps1)
                        rope(q01, None, None, slice(0, QB))
                        rope(k01, None, None, slice(0, QB))
                        for i in range(5):
                            run_unit(i, units[i])
                    # scope B: pp's banks become the wo / late-vproj ring
                    with tc.tile_pool(name="wp", bufs=1, space="PSUM") as wpp:
                        pools["wp"] = wpp
                        pool_tag[id(wpp)] = "wp"
                        for i in range(5, len(units)):
                            run_unit(i, units[i])
                        for p in (state["fin"] + state["fin_hold"]
                                  + state["pv"]):
                            p()

    nc.compile()
    return nc


def _host_prep(x, Wqkv, Wo, seq_len):
    import ml_dtypes
    bf16 = ml_dtypes.bfloat16
    x = np.asarray(x, dtype=np.float32)
    Wqkv = np.asarray(Wqkv, dtype=np.float32)
    Wo = np.asarray(Wo, dtype=np.float32)
    off = int(np.asarray(seq_len).reshape(()))

    inv = 1.0 / (10000.0 ** (np.arange(0, DH, 2, dtype=np.float64) / DH))  # [32]
    pos = np.arange(T, dtype=np.float64) + off
    ang = pos[:, None] * inv[None, :]                 # [T, 32]
    cs = np.cos(ang).T                                # [32, T]
    sn = np.sin(ang).T
    tb = np.empty((128, 2 * T), np.float32)
    for blk in range(2):
        r0 = blk * 64
        tb[r0:r0 + 32, 0:T] = cs
        tb[r0 + 32:r0 + 64, 0:T] = cs
        # sin rows swapped + sign-folded: row s holds the coefficient X[s]
        # is multiplied by when producing output row s^32 (see rope()).
        tb[r0:r0 + 32, T:2 * T] = sn
        tb[r0 + 32:r0 + 64, T:2 * T] = -sn

    in_maps = []
    for core in range(NC_):
        b, g = core // 4, core % 4
        hs = [3 * g, 3 * g + 1, 3 * g + 2]
        q = [Wqkv[:, h * DH:(h + 1) * DH] for h in hs]
        k = [Wqkv[:, C + h * DH:C + (h + 1) * DH] for h in hs]
        v = [Wqkv[:, 2 * C + h * DH:2 * C + (h + 1) * DH] for h in hs]
        wqkv_l = np.concatenate(
            [q[0], q[1], k[0], k[1], q[2], k[2], v[0], v[1], v[2]], axis=1)
        # xp[p, k*T + t] = x[b][t, k*128 + p]
        xp = np.ascontiguousarray(
            x[b].T.reshape(6, 128, T).transpose(1, 0, 2).reshape(128, 6 * T))
        # wqp[p, k*576 + j] = wqkv_l[k*128 + p, j]
        wqp = np.ascontiguousarray(
            wqkv_l.reshape(6, 128, 576).transpose(1, 0, 2).reshape(128, 6 * 576))
        wo_l = Wo[g * HPC * DH:(g + 1) * HPC * DH, :]   # [192, 768]
        wop = np.zeros((128, 1536), np.float32)
        wop[:, 0:768] = wo_l[0:128]
        wop[0:64, 768:1536] = wo_l[128:192]
        in_maps.append({
            "xp": xp.astype(bf16),
            "wqp": wqp.astype(bf16),
            "tbp": tb.astype(bf16),
            "wop": wop.astype(bf16),
        })
    return in_maps


def _run(in_maps, trace=False):
    global _prog
    from concourse.bass_utils import run_bass_kernel_spmd
    if _prog is None:
        _prog = _build()
    return run_bass_kernel_spmd(_prog, in_maps, list(range(NC_)), trace=trace)


def kernel(x, Wqkv, Wo, seq_len):
    in_maps = _host_prep(x, Wqkv, Wo, seq_len)
    res = _run(in_maps, trace=False)
    out = np.zeros((B, T, C), dtype=np.float32)
    for core in range(NC_):
        out[core // 4] += np.asarray(res.results[core]["out"], dtype=np.float32)
    return out
